# revision 13
# baseline (speedup 1.0000x reference)
"""Bass/Trainium2 kernel for nn_DotProductAttention_47528108097846.

reference:
    scores = einsum('bhqd,bhkd->bhqk', Q, K) / 16
    attn = softmax(scores, axis=-1)
    h = einsum('bhqk,bhkd->bhqd', attn, V)
    return reshape(h, (S, B, H, D))

B=2, H=8, S=4096, D=64. 16 (b,h) pairs sharded as 2 per NeuronCore across 8
cores (batch+head parallel, no cross-core comms).

Per-core algorithm (2 heads), all matmuls bf16, fp32 PSUM accumulation:

  prologue (DMA/xbar only -- no PE, no DVE):
  - Q/K/V loaded with gpsimd-initiated casting DMAs (fp32 DRAM -> bf16
    SBUF), 128-row blocks in partitions.
  - K: per pair of 128-key blocks (2p, 2p+1), one XBAR dma_start_transpose
    of the [128, 128] pair view yields kt[:, p, :]: rows 0:64 = D dims of
    block 2p, rows 64:128 = D dims of block 2p+1 -- a full-128-contraction
    weight tile serving both blocks.
  - Q: same XBAR, then split by parity into qt_lo (even q-blocks in rows
    0:64, rows 64:128 zero) and qt_hi (odd q-blocks in rows 64:128, rows
    0:64 zero). The zero half makes the unwanted parity of the K pair-tile
    contract to 0.
  - V' = [V | 1 | 0pad] [128, 128] per block (ones column 64 gives the
    softmax denominator for free in the AV matmul).

  main loop, per q-group (1024 q) x k-block (128 keys):
  - scoresT j-half [128, 512] = (lhsT=kt[:, kb//2, :]).T @ (rhs = qt_lo or
    qt_hi 4-block slice).  j=0 streams the 4 even q-blocks, j=1 the 4 odd
    ones (a per-q-group column permutation undone by the output DMA).
  - exp: bank A (j=0) on ScalarE (exp activation, scale 1/16, bias
    16*ln(d0)); bank B (j=1) on the DVE via a custom 8-stage op
    ((c0*s+c1)*s+1)^16 ~= exp(s/16)/d0^16 (deg-2 fit + 4 squarings).
    Softmax cancels the shared d0^16.  Separate single-bank PSUM pools keep
    the two QK->exp->QK chains independent.
  - outT [128,1024] += (lhsT=V'_kb).T @ expT, software-pipelined at depth 2
    (AV of kb-2 interleaves QK of kb) so exp latency hides behind ~1.7us of
    in-order PE work.
  - epilogue: copy outT[0:65] to SBUF (DVE), PE-transpose [65,128] strips,
    reciprocal of the denominator column (DVE), scale-by-reciprocal
    (ScalarE activation Copy), permuted-destination DMA back to DRAM.
"""
import numpy as np

import concourse.bass as bass
import concourse.bacc as bacc
import concourse.tile as tile
from concourse import mybir
from concourse.masks import make_identity
from concourse.bass_utils import run_bass_kernel_spmd

B, H, S, D = 2, 8, 4096, 64
N_CORES = 8
PAIRS_PER_CORE = (B * H) // N_CORES  # 2 heads per core

f32 = mybir.dt.float32
bf16 = mybir.dt.bfloat16

QG = 1024            # q-group width
NQG = S // QG        # 4 q-groups per head
NKB = S // 128       # 32 k-blocks per head
NPB = NKB // 2       # 16 block pairs

# ---------------------------------------------------------------------------
# Custom DVE op: EXP16 -- out = ((c0*s + c1)*s + 1)^16 ~= exp(s/16)/d0^16.
# Deg-2 least-squares fit of e^u/d0 on u = s/256 in [-0.22, 0.22] (covers
# |s| <= 56; randn scores have sigma = 8).
EXP16_NAME = "EXP16_POLY_ANT"
EXP_D0 = 1.0000875648796109
EXP_E1 = 1.0070340603478836
EXP_E2 = 0.49672662859727144
EXP_C0 = float(EXP_E2 / 256.0**2)
EXP_C1 = float(EXP_E1 / 256.0)
EXP_BIAS = float(16.0 * np.log(EXP_D0))


def _np_exp16(in0, in1, s0, s1, imm2):
    q = (in0.astype(np.float32) * s0 + s1) * in0 + 1.0
    q = q * q
    q = q * q
    q = q * q
    return q * q


def register_exp16():
    import concourse.dve_ops as dve_ops_mod
    from concourse.dve_ops import DveOp
    from concourse.dve_spec import C0, C1, One, Spec, Src0, lower, _has_src1
    from concourse.dve_uop import DveOpSpec

    for op in dve_ops_mod.OPS:
        if op.name == EXP16_NAME:
            return op
    m1 = Src0 * C0
    a1 = m1 + C1
    m2 = a1 * Src0
    a2 = m2 + One
    y1 = a2 * a2
    y2 = y1 * y1
    y3 = y2 * y2
    y4 = y3 * y3
    spec = Spec(body=y4, reference=_np_exp16)
    row = dve_ops_mod._CUSTOM_DVE_ROW_BASE + len(dve_ops_mod.OPS)
    assert row < 0x20, "no free custom-DVE rows"
    dve_ops_mod._SUB_OPCODE_FOR_NAME[EXP16_NAME] = row
    shas = {}
    for ver in ("v3", "v4"):
        try:
            uops = lower(spec, ver=ver)
        except Exception:
            continue
        shas[ver] = DveOpSpec(
            name=EXP16_NAME, opcode=row, uops=uops, rd1_en=_has_src1(spec)
        ).sha(ver)
    op = DveOp(EXP16_NAME, spec, subdim=False, uops_sha=shas)
    dve_ops_mod.OPS.append(op)
    dve_ops_mod.CUSTOM_DVE_SPECS[EXP16_NAME] = spec
    return op


def build_attention(nc, tc, q, k, v, o, repeat_loop=None, mode="full"):
    """Emit attention for PAIRS_PER_CORE heads.

    q/k/v/o: DRAM APs of shape [PAIRS_PER_CORE, S, D] (fp32).
    """
    import contextlib
    exp16 = register_exp16()
    ctx = contextlib.ExitStack()
    consts = ctx.enter_context(tc.tile_pool(name="consts", bufs=1))
    nat = ctx.enter_context(tc.tile_pool(name="nat", bufs=2))
    persist = ctx.enter_context(tc.tile_pool(name="persist", bufs=1))
    sb = ctx.enter_context(tc.tile_pool(name="sb", bufs=3))
    pool_e = ctx.enter_context(tc.tile_pool(name="sb_e", bufs=6))
    # two independent single-bank score pools: PSUM pool rotation is
    # tile-granular, so bank A's reuse must not wait on bank B's reader
    pool_sA = ctx.enter_context(tc.tile_pool(name="ps_sA", bufs=2, space="PSUM"))
    pool_sB = ctx.enter_context(tc.tile_pool(name="ps_sB", bufs=2, space="PSUM"))
    pool_o = ctx.enter_context(tc.tile_pool(name="ps_o", bufs=1, space="PSUM"))
    pool_t = ctx.enter_context(tc.tile_pool(name="ps_t", bufs=2, space="PSUM"))

    if mode == "copyonly":
        for h in range(PAIRS_PER_CORE):
            t = None
            for src in (q, k, v):
                t = nat.tile([128, NKB, 64], f32, tag="nat")
                nc.sync.dma_start(
                    out=t, in_=src[h].rearrange("(n p) d -> p n d", p=128))
            nc.sync.dma_start(
                out=o[h].rearrange("(n p) d -> p n d", p=128), in_=t)
        ctx.close()
        return

    # exp bias for ScalarE (matches the DVE poly's d0^16 scale) + act-table
    # preload off the critical path
    bias_ap = consts.tile([128, 1], f32)
    nc.vector.memset(bias_ap, EXP_BIAS)
    dummy = consts.tile([128, 1], f32)
    nc.vector.memset(dummy, 0.0)
    dummy_o = consts.tile([128, 1], bf16)
    nc.scalar.activation(out=dummy_o, in_=dummy,
                         func=mybir.ActivationFunctionType.Exp,
                         bias=bias_ap, scale=1.0 / 16.0)

    ident = consts.tile([128, 128], f32)
    make_identity(nc, ident)

    # ---------------- prologue: casting DMAs + XBAR transposes ----------
    # Four zero-padded Q layouts: the kt pair tile holds K block 2p in
    # contraction rows 0:64 and block 2p+1 in rows 64:128, so scoring
    # k-block parity pk against q-parity pq needs the q content in rows
    # [64*pk : 64*pk+64] with the other half zero:
    #   ql_e: even q @ rows 0:64   ql_o: odd q @ rows 0:64
    #   qh_e: even q @ rows 64:128 qh_o: odd q @ rows 64:128
    qles, qlos_, qhes, qhos, kts, v1s = [], [], [], [], [], []

    def emit_prologue(h):
        kt = persist.tile([128, NPB, 128], bf16, tag=f"kt{h}")
        ql_e = persist.tile([128, NPB, 128], bf16, tag=f"qle{h}")
        ql_o = persist.tile([128, NPB, 128], bf16, tag=f"qlo{h}")
        qh_e = persist.tile([128, NPB, 128], bf16, tag=f"qhe{h}")
        qh_o = persist.tile([128, NPB, 128], bf16, tag=f"qho{h}")
        v1 = persist.tile([128, NKB, 128], bf16, tag=f"v1{h}")
        kts.append(kt)
        qles.append(ql_e)
        qlos_.append(ql_o)
        qhes.append(qh_e)
        qhos.append(qh_o)
        v1s.append(v1)
        # zero parity halves (no other writer touches them)
        nc.gpsimd.memset(ql_e[64:128], 0.0)
        nc.gpsimd.memset(ql_o[64:128], 0.0)
        nc.gpsimd.memset(qh_e[0:64], 0.0)
        nc.gpsimd.memset(qh_o[0:64], 0.0)

        CH = 8                      # blocks per load chunk
        NP_CH = CH // 2             # pairs per chunk
        for g in range(NKB // CH):
            for (src, tag) in ((k, "k"), (q, "q")):
                natb = nat.tile([128, CH, 64], bf16, tag="natb")
                nc.gpsimd.dma_start(
                    out=natb,
                    in_=src[h].rearrange("(n p) d -> p n d", p=128)[
                        :, g * CH:(g + 1) * CH, :])
                if tag == "k":
                    for p in range(NP_CH):
                        nc.sync.dma_start_transpose(
                            out=kt[:, g * NP_CH + p, :],
                            in_=natb[:, 2 * p:2 * p + 2, :])
                else:
                    qtp = nat.tile([128, NP_CH, 128], bf16, tag="qtp")
                    for p in range(NP_CH):
                        nc.sync.dma_start_transpose(
                            out=qtp[:, p, :],
                            in_=natb[:, 2 * p:2 * p + 2, :])
                    gs = slice(g * NP_CH, (g + 1) * NP_CH)
                    # same-partition copies on gpsimd
                    nc.gpsimd.tensor_copy(
                        out=ql_e[0:64, gs, :], in_=qtp[0:64])
                    nc.gpsimd.tensor_copy(
                        out=qh_o[64:128, gs, :], in_=qtp[64:128])
                    # cross-partition moves via DMA (engines cannot)
                    nc.sync.dma_start(
                        out=ql_o[0:64, gs, :], in_=qtp[64:128])
                    nc.sync.dma_start(
                        out=qh_e[64:128, gs, :], in_=qtp[0:64])
            if g == 1:
                # V' build: casting DMA straight into columns 0:64; ones
                # column + zero pad via memset
                nc.gpsimd.memset(v1[:, :, 64:65], 1.0)
                nc.gpsimd.memset(v1[:, :, 65:128], 0.0)
                nc.gpsimd.dma_start(
                    out=v1[:, :, 0:64],
                    in_=v[h].rearrange("(n p) d -> p n d", p=128))

    emit_prologue(0)
    defer_prologues = (repeat_loop is None and mode == "full")
    if not defer_prologues:
        for h in range(1, PAIRS_PER_CORE):
            emit_prologue(h)

    # ---------------- main loops --------------------------------------
    def main_compute():
        for h in range(PAIRS_PER_CORE):
            kt, v1 = kts[h], v1s[h]
            ql_e, ql_o, qh_e, qh_o = qles[h], qlos_[h], qhes[h], qhos[h]
            out_r = o[h].rearrange("(n p) d -> p n d", p=128)
            for qg in range(NQG):
                ps_o = pool_o.tile([128, QG], f32, tag="o")

                def av(prev_eT, prev_kb, j):
                    nc.tensor.matmul(
                        out=ps_o[:, j * 512:(j + 1) * 512],
                        lhsT=v1[:, prev_kb, :],
                        rhs=prev_eT[:, j * 512:(j + 1) * 512],
                        start=(prev_kb == 0), stop=(prev_kb == NKB - 1))

                # software-pipelined at depth 2: QK(kb) interleaves with the
                # accumulating AV matmuls of kb-2 (the PE runs its queue in
                # order, so AV readiness must trail by ~2 tiles of PE work)
                pend = []
                for kb in range(NKB):
                    ps_sa = pool_sA.tile([128, 512], f32, tag="sA")
                    ps_sb = pool_sB.tile([128, 512], f32, tag="sB")
                    ps_sj = (ps_sa, ps_sb)
                    eT = pool_e.tile([128, QG], bf16, tag="exp")
                    qj = (ql_e, ql_o) if kb % 2 == 0 else (qh_e, qh_o)
                    for j in range(QG // 512):
                        nc.tensor.matmul(
                            out=ps_sj[j],
                            lhsT=kt[:, kb // 2, :],
                            rhs=qj[j][:, 4 * qg:4 * qg + 4, :],
                            start=True, stop=True)
                        # exp of this half right after its producing matmul:
                        # bank A (even q-blocks) -> ScalarE, bank B (odd) ->
                        # DVE custom op
                        if j == 0:
                            nc.scalar.activation(
                                out=eT[:, 0:512], in_=ps_sj[0],
                                func=mybir.ActivationFunctionType.Exp,
                                bias=bias_ap, scale=1.0 / 16.0)
                        else:
                            nc.vector._custom_dve(
                                exp16, out=eT[:, 512:QG], in0=ps_sj[1],
                                s0=EXP_C0, s1=EXP_C1)
                        if len(pend) >= 2:
                            av(pend[0][0], pend[0][1], j)
                            if j == QG // 512 - 1:
                                pend.pop(0)
                    pend.append((eT, kb))
                for eT_p, kb_p in pend:
                    for j in range(QG // 512):
                        av(eT_p, kb_p, j)

                # epilogue for this q-group.  ps_o columns hold q-blocks in
                # parity-permuted order [0,2,4,6,1,3,5,7]; the DMA dest APs
                # restore natural order.
                oT = sb.tile([65, QG], f32, tag="oT")
                nc.vector.tensor_copy(out=oT, in_=ps_o[0:65, :])
                out_sb = sb.tile([128, QG // 128, 64], f32, tag="out")
                # dest AP [p, u, t, d]: (u, t) -> physical q-block 2t+u, so
                # iterating u-major hits [0,2,4,6,1,3,5,7] -- undoing the
                # parity permutation of out_sb's i index (i = u*4+t)
                dst8 = out_r[:, qg * 8:(qg + 1) * 8, :].rearrange(
                    "p (t u) d -> p u t d", t=4, u=2)
                src8 = None  # filled per half below
                for i in range(QG // 128):
                    ps_t = pool_t.tile([128, 65], f32, tag="t")
                    nc.tensor.transpose(
                        ps_t, oT[:, i * 128:(i + 1) * 128],
                        ident[0:65, 0:65])
                    rcp = sb.tile([128, 1], f32, tag="rcp")
                    nc.vector.reciprocal(out=rcp, in_=ps_t[:, 64:65])
                    nc.scalar.activation(
                        out=out_sb[:, i, :], in_=ps_t[:, 0:64],
                        func=mybir.ActivationFunctionType.Copy,
                        scale=rcp)
                    if i == 3:
                        nc.sync.dma_start(
                            out=dst8[:, 0, :, :], in_=out_sb[:, 0:4, :])
                nc.sync.dma_start(
                    out=dst8[:, 1, :, :], in_=out_sb[:, 4:8, :])
                if defer_prologues and h == 0 and qg == 0:
                    for h2 in range(1, PAIRS_PER_CORE):
                        emit_prologue(h2)

    if mode == "prologue":
        pass
    elif repeat_loop is None:
        main_compute()
    else:
        with tc.For_i(0, repeat_loop, 1) as _:
            main_compute()
    ctx.close()


_CACHED = {}


def build_program(repeat_loop=None, mode="full"):
    key = (repeat_loop, mode)
    if key in _CACHED:
        return _CACHED[key]
    nc = bacc.Bacc("TRN2", target_bir_lowering=False, debug=False,
                   num_devices=N_CORES)
    q = nc.dram_tensor("q", [PAIRS_PER_CORE, S, D], f32,
                       kind="ExternalInput").ap()
    k = nc.dram_tensor("k", [PAIRS_PER_CORE, S, D], f32,
                       kind="ExternalInput").ap()
    v = nc.dram_tensor("v", [PAIRS_PER_CORE, S, D], f32,
                       kind="ExternalInput").ap()
    o = nc.dram_tensor("o", [PAIRS_PER_CORE, S, D], f32,
                       kind="ExternalOutput").ap()
    with tile.TileContext(nc) as tc:
        build_attention(nc, tc, q, k, v, o, repeat_loop=repeat_loop,
                        mode=mode)
    nc.compile()
    _CACHED[key] = nc
    return nc


def kernel(queries, keys, values, adj=None, **_unused):
    """Full-input attention on 8 NeuronCores. Returns [S, B, H, D] fp32."""
    queries = np.ascontiguousarray(queries, dtype=np.float32)
    keys = np.ascontiguousarray(keys, dtype=np.float32)
    values = np.ascontiguousarray(values, dtype=np.float32)

    nc = build_program()
    qf = queries.reshape(B * H, S, D)
    kf = keys.reshape(B * H, S, D)
    vf = values.reshape(B * H, S, D)
    in_maps = []
    for c in range(N_CORES):
        sl = slice(c * PAIRS_PER_CORE, (c + 1) * PAIRS_PER_CORE)
        in_maps.append({"q": qf[sl], "k": kf[sl], "v": vf[sl]})
    res = run_bass_kernel_spmd(nc, in_maps, list(range(N_CORES)))
    hout = np.empty((B * H, S, D), dtype=np.float32)
    for c in range(N_CORES):
        hout[c * PAIRS_PER_CORE:(c + 1) * PAIRS_PER_CORE] = res.results[c]["o"]
    return hout.reshape(B, H, S, D).reshape(S, B, H, D)


# revision 17
# speedup vs baseline: 1.0159x; 1.0159x over previous
"""Bass/Trainium2 kernel for nn_DotProductAttention_47528108097846.

reference:
    scores = einsum('bhqd,bhkd->bhqk', Q, K) / 16
    attn = softmax(scores, axis=-1)
    h = einsum('bhqk,bhkd->bhqd', attn, V)
    return reshape(h, (S, B, H, D))

B=2, H=8, S=4096, D=64. 16 (b,h) pairs sharded as 2 per NeuronCore across 8
cores (batch+head parallel, no cross-core comms).

Per-core algorithm (2 heads), all matmuls bf16, fp32 PSUM accumulation:

  prologue (DMA/xbar only -- no PE, no DVE):
  - Q/K/V loaded with gpsimd-initiated casting DMAs (fp32 DRAM -> bf16
    SBUF), 128-row blocks in partitions.
  - K: per pair of 128-key blocks (2p, 2p+1), one XBAR dma_start_transpose
    of the [128, 128] pair view yields kt[:, p, :]: rows 0:64 = D dims of
    block 2p, rows 64:128 = D dims of block 2p+1 -- a full-128-contraction
    weight tile serving both blocks.
  - Q: same XBAR, then split by parity into qt_lo (even q-blocks in rows
    0:64, rows 64:128 zero) and qt_hi (odd q-blocks in rows 64:128, rows
    0:64 zero). The zero half makes the unwanted parity of the K pair-tile
    contract to 0.
  - V' = [V | 1 | 0pad] [128, 128] per block (ones column 64 gives the
    softmax denominator for free in the AV matmul).

  main loop, per q-group (1024 q) x k-block (128 keys):
  - scoresT j-half [128, 512] = (lhsT=kt[:, kb//2, :]).T @ (rhs = qt_lo or
    qt_hi 4-block slice).  j=0 streams the 4 even q-blocks, j=1 the 4 odd
    ones (a per-q-group column permutation undone by the output DMA).
  - exp: bank A (j=0) on ScalarE (exp activation, scale 1/16, bias
    16*ln(d0)); bank B (j=1) on the DVE via a custom 8-stage op
    ((c0*s+c1)*s+1)^16 ~= exp(s/16)/d0^16 (deg-2 fit + 4 squarings).
    Softmax cancels the shared d0^16.  Separate single-bank PSUM pools keep
    the two QK->exp->QK chains independent.
  - outT [128,1024] += (lhsT=V'_kb).T @ expT, software-pipelined at depth 2
    (AV of kb-2 interleaves QK of kb) so exp latency hides behind ~1.7us of
    in-order PE work.
  - epilogue: copy outT[0:65] to SBUF (DVE), PE-transpose [65,128] strips,
    reciprocal of the denominator column (DVE), scale-by-reciprocal
    (ScalarE activation Copy), permuted-destination DMA back to DRAM.
"""
import numpy as np

import concourse.bass as bass
import concourse.bacc as bacc
import concourse.tile as tile
from concourse import mybir
from concourse.masks import make_identity
from concourse.bass_utils import run_bass_kernel_spmd

B, H, S, D = 2, 8, 4096, 64
N_CORES = 8
PAIRS_PER_CORE = (B * H) // N_CORES  # 2 heads per core

f32 = mybir.dt.float32
bf16 = mybir.dt.bfloat16

QG = 1024            # q-group width
NQG = S // QG        # 4 q-groups per head
NKB = S // 128       # 32 k-blocks per head
NPB = NKB // 2       # 16 block pairs

# ---------------------------------------------------------------------------
# Custom DVE op: EXP16 -- out = ((c0*s + c1)*s + 1)^16 ~= exp(s/16)/d0^16.
# Deg-2 least-squares fit of e^u/d0 on u = s/256 in [-0.22, 0.22] (covers
# |s| <= 56; randn scores have sigma = 8).
EXP16_NAME = "EXP16_POLY_ANT"
EXP_D0 = 1.0000875648796109
EXP_E1 = 1.0070340603478836
EXP_E2 = 0.49672662859727144
EXP_C0 = float(EXP_E2 / 256.0**2)
EXP_C1 = float(EXP_E1 / 256.0)
EXP_BIAS = float(16.0 * np.log(EXP_D0))


def _np_exp16(in0, in1, s0, s1, imm2):
    q = (in0.astype(np.float32) * s0 + s1) * in0 + 1.0
    q = q * q
    q = q * q
    q = q * q
    return q * q


def register_exp16():
    import concourse.dve_ops as dve_ops_mod
    from concourse.dve_ops import DveOp
    from concourse.dve_spec import C0, C1, One, Spec, Src0, lower, _has_src1
    from concourse.dve_uop import DveOpSpec

    for op in dve_ops_mod.OPS:
        if op.name == EXP16_NAME:
            return op
    m1 = Src0 * C0
    a1 = m1 + C1
    m2 = a1 * Src0
    a2 = m2 + One
    y1 = a2 * a2
    y2 = y1 * y1
    y3 = y2 * y2
    y4 = y3 * y3
    spec = Spec(body=y4, reference=_np_exp16)
    row = dve_ops_mod._CUSTOM_DVE_ROW_BASE + len(dve_ops_mod.OPS)
    assert row < 0x20, "no free custom-DVE rows"
    dve_ops_mod._SUB_OPCODE_FOR_NAME[EXP16_NAME] = row
    shas = {}
    for ver in ("v3", "v4"):
        try:
            uops = lower(spec, ver=ver)
        except Exception:
            continue
        shas[ver] = DveOpSpec(
            name=EXP16_NAME, opcode=row, uops=uops, rd1_en=_has_src1(spec)
        ).sha(ver)
    op = DveOp(EXP16_NAME, spec, subdim=False, uops_sha=shas)
    dve_ops_mod.OPS.append(op)
    dve_ops_mod.CUSTOM_DVE_SPECS[EXP16_NAME] = spec
    return op


def build_attention(nc, tc, q, k, v, o, repeat_loop=None, mode="full"):
    """Emit attention for PAIRS_PER_CORE heads.

    q/k/v/o: DRAM APs of shape [PAIRS_PER_CORE, S, D] (fp32).
    """
    import contextlib
    exp16 = register_exp16()
    ctx = contextlib.ExitStack()
    consts = ctx.enter_context(tc.tile_pool(name="consts", bufs=1))
    nat = ctx.enter_context(tc.tile_pool(name="nat", bufs=2))
    natk = ctx.enter_context(tc.tile_pool(name="natk", bufs=2))
    natq = ctx.enter_context(tc.tile_pool(name="natq", bufs=2))
    natp = ctx.enter_context(tc.tile_pool(name="natp", bufs=2))
    persist = ctx.enter_context(tc.tile_pool(name="persist", bufs=1))
    sb = ctx.enter_context(tc.tile_pool(name="sb", bufs=3))
    pool_e = ctx.enter_context(tc.tile_pool(name="sb_e", bufs=6))
    # two independent single-bank score pools: PSUM pool rotation is
    # tile-granular, so bank A's reuse must not wait on bank B's reader
    pool_sA = ctx.enter_context(tc.tile_pool(name="ps_sA", bufs=2, space="PSUM"))
    pool_sB = ctx.enter_context(tc.tile_pool(name="ps_sB", bufs=2, space="PSUM"))
    pool_o = ctx.enter_context(tc.tile_pool(name="ps_o", bufs=1, space="PSUM"))
    pool_t = ctx.enter_context(tc.tile_pool(name="ps_t", bufs=2, space="PSUM"))

    if mode == "copyonly":
        for h in range(PAIRS_PER_CORE):
            t = None
            for src in (q, k, v):
                t = nat.tile([128, NKB, 64], f32, tag="nat")
                nc.sync.dma_start(
                    out=t, in_=src[h].rearrange("(n p) d -> p n d", p=128))
            nc.sync.dma_start(
                out=o[h].rearrange("(n p) d -> p n d", p=128), in_=t)
        ctx.close()
        return

    # exp bias for ScalarE (matches the DVE poly's d0^16 scale) + act-table
    # preload off the critical path
    bias_ap = consts.tile([128, 1], f32)
    nc.vector.memset(bias_ap, EXP_BIAS)
    dummy = consts.tile([128, 1], f32)
    nc.vector.memset(dummy, 0.0)
    dummy_o = consts.tile([128, 1], bf16)
    nc.scalar.activation(out=dummy_o, in_=dummy,
                         func=mybir.ActivationFunctionType.Exp,
                         bias=bias_ap, scale=1.0 / 16.0)

    ident = consts.tile([128, 128], f32)
    make_identity(nc, ident)

    # ---------------- prologue: casting DMAs + XBAR transposes ----------
    # Four zero-padded Q layouts: the kt pair tile holds K block 2p in
    # contraction rows 0:64 and block 2p+1 in rows 64:128, so scoring
    # k-block parity pk against q-parity pq needs the q content in rows
    # [64*pk : 64*pk+64] with the other half zero:
    #   ql_e: even q @ rows 0:64   ql_o: odd q @ rows 0:64
    #   qh_e: even q @ rows 64:128 qh_o: odd q @ rows 64:128
    qles, qlos_, qhes, qhos, kts, v1s = [], [], [], [], [], []

    def emit_prologue(h):
        kt = persist.tile([128, NPB, 128], bf16, tag=f"kt{h}")
        ql_e = persist.tile([128, NPB, 128], bf16, tag=f"qle{h}")
        ql_o = persist.tile([128, NPB, 128], bf16, tag=f"qlo{h}")
        qh_e = persist.tile([128, NPB, 128], bf16, tag=f"qhe{h}")
        qh_o = persist.tile([128, NPB, 128], bf16, tag=f"qho{h}")
        v1 = persist.tile([128, NKB, 128], bf16, tag=f"v1{h}")
        kts.append(kt)
        qles.append(ql_e)
        qlos_.append(ql_o)
        qhes.append(qh_e)
        qhos.append(qh_o)
        v1s.append(v1)
        # zero parity halves (no other writer touches them)
        nc.gpsimd.memset(ql_e[64:128], 0.0)
        nc.gpsimd.memset(ql_o[64:128], 0.0)
        nc.gpsimd.memset(qh_e[0:64], 0.0)
        nc.gpsimd.memset(qh_o[0:64], 0.0)

        CH = 8                      # blocks per load chunk
        NP_CH = CH // 2             # pairs per chunk
        for g in range(NKB // CH):
            gs = slice(g * NP_CH, (g + 1) * NP_CH)
            natbk = natk.tile([128, CH, 64], bf16, tag="natbk")
            nc.gpsimd.dma_start(
                out=natbk,
                in_=k[h].rearrange("(n p) d -> p n d", p=128)[
                    :, g * CH:(g + 1) * CH, :])
            # one batched XBAR: NP_CH pair-transposes per instruction
            nc.sync.dma_start_transpose(out=kt[:, gs, :], in_=natbk)
            natbq = natq.tile([128, CH, 64], bf16, tag="natbq")
            nc.gpsimd.dma_start(
                out=natbq,
                in_=q[h].rearrange("(n p) d -> p n d", p=128)[
                    :, g * CH:(g + 1) * CH, :])
            qtp = natp.tile([128, NP_CH, 128], bf16, tag="qtp")
            nc.sync.dma_start_transpose(out=qtp, in_=natbq)
            # same-partition copies on gpsimd
            nc.gpsimd.tensor_copy(out=ql_e[0:64, gs, :], in_=qtp[0:64])
            nc.gpsimd.tensor_copy(out=qh_o[64:128, gs, :], in_=qtp[64:128])
            # cross-partition moves via DMA (engines cannot)
            nc.sync.dma_start(out=ql_o[0:64, gs, :], in_=qtp[64:128])
            nc.sync.dma_start(out=qh_e[64:128, gs, :], in_=qtp[0:64])
            if g == 1:
                # V' build: casting DMA straight into columns 0:64; ones
                # column + zero pad via memset
                nc.gpsimd.memset(v1[:, :, 64:65], 1.0)
                nc.gpsimd.memset(v1[:, :, 65:128], 0.0)
                nc.gpsimd.dma_start(
                    out=v1[:, :, 0:64],
                    in_=v[h].rearrange("(n p) d -> p n d", p=128))

    # all prologues upfront: they only use gpsimd + SP + DMA engines, which
    # the main loop leaves idle (out-DMAs are issued from ScalarE)
    for h in range(PAIRS_PER_CORE):
        emit_prologue(h)
    defer_prologues = False

    # ---------------- main loops --------------------------------------
    def main_compute():
        for h in range(PAIRS_PER_CORE):
            kt, v1 = kts[h], v1s[h]
            ql_e, ql_o, qh_e, qh_o = qles[h], qlos_[h], qhes[h], qhos[h]
            out_r = o[h].rearrange("(n p) d -> p n d", p=128)
            for qg in range(NQG):
                ps_o = pool_o.tile([128, QG], f32, tag="o")

                def av(prev_eT, prev_kb, j):
                    nc.tensor.matmul(
                        out=ps_o[:, j * 512:(j + 1) * 512],
                        lhsT=v1[:, prev_kb, :],
                        rhs=prev_eT[:, j * 512:(j + 1) * 512],
                        start=(prev_kb == 0), stop=(prev_kb == NKB - 1))

                # software-pipelined at depth 2: QK(kb) interleaves with the
                # accumulating AV matmuls of kb-2 (the PE runs its queue in
                # order, so AV readiness must trail by ~2 tiles of PE work)
                pend = []
                for kb in range(NKB):
                    ps_sa = pool_sA.tile([128, 512], f32, tag="sA")
                    ps_sb = pool_sB.tile([128, 512], f32, tag="sB")
                    ps_sj = (ps_sa, ps_sb)
                    eT = pool_e.tile([128, QG], bf16, tag="exp")
                    qj = (ql_e, ql_o) if kb % 2 == 0 else (qh_e, qh_o)
                    for j in range(QG // 512):
                        nc.tensor.matmul(
                            out=ps_sj[j],
                            lhsT=kt[:, kb // 2, :],
                            rhs=qj[j][:, 4 * qg:4 * qg + 4, :],
                            start=True, stop=True)
                        # exp of this half right after its producing matmul:
                        # bank A (even q-blocks) -> ScalarE, bank B (odd) ->
                        # DVE custom op
                        if j == 0:
                            nc.scalar.activation(
                                out=eT[:, 0:512], in_=ps_sj[0],
                                func=mybir.ActivationFunctionType.Exp,
                                bias=bias_ap, scale=1.0 / 16.0)
                        else:
                            nc.vector._custom_dve(
                                exp16, out=eT[:, 512:QG], in0=ps_sj[1],
                                s0=EXP_C0, s1=EXP_C1)
                        if len(pend) >= 2:
                            av(pend[0][0], pend[0][1], j)
                            if j == QG // 512 - 1:
                                pend.pop(0)
                    pend.append((eT, kb))
                for eT_p, kb_p in pend:
                    for j in range(QG // 512):
                        av(eT_p, kb_p, j)

                # epilogue for this q-group.  ps_o columns hold q-blocks in
                # parity-permuted order [0,2,4,6,1,3,5,7]; the DMA dest APs
                # restore natural order.
                oT = sb.tile([65, QG], f32, tag="oT")
                nc.vector.tensor_copy(out=oT, in_=ps_o[0:65, :])
                out_sb = sb.tile([128, QG // 128, 64], f32, tag="out")
                # dest AP [p, u, t, d]: (u, t) -> physical q-block 2t+u, so
                # iterating u-major hits [0,2,4,6,1,3,5,7] -- undoing the
                # parity permutation of out_sb's i index (i = u*4+t)
                dst8 = out_r[:, qg * 8:(qg + 1) * 8, :].rearrange(
                    "p (t u) d -> p u t d", t=4, u=2)
                src8 = None  # filled per half below
                for i in range(QG // 128):
                    ps_t = pool_t.tile([128, 65], f32, tag="t")
                    nc.tensor.transpose(
                        ps_t, oT[:, i * 128:(i + 1) * 128],
                        ident[0:65, 0:65])
                    rcp = sb.tile([128, 1], f32, tag="rcp")
                    nc.vector.reciprocal(out=rcp, in_=ps_t[:, 64:65])
                    nc.scalar.activation(
                        out=out_sb[:, i, :], in_=ps_t[:, 0:64],
                        func=mybir.ActivationFunctionType.Copy,
                        scale=rcp)
                    if i == 3:
                        nc.scalar.dma_start(
                            out=dst8[:, 0, :, :], in_=out_sb[:, 0:4, :])
                nc.scalar.dma_start(
                    out=dst8[:, 1, :, :], in_=out_sb[:, 4:8, :])

    if mode == "prologue":
        pass
    elif repeat_loop is None:
        main_compute()
    else:
        with tc.For_i(0, repeat_loop, 1) as _:
            main_compute()
    ctx.close()


_CACHED = {}


def build_program(repeat_loop=None, mode="full"):
    key = (repeat_loop, mode)
    if key in _CACHED:
        return _CACHED[key]
    nc = bacc.Bacc("TRN2", target_bir_lowering=False, debug=False,
                   num_devices=N_CORES)
    q = nc.dram_tensor("q", [PAIRS_PER_CORE, S, D], f32,
                       kind="ExternalInput").ap()
    k = nc.dram_tensor("k", [PAIRS_PER_CORE, S, D], f32,
                       kind="ExternalInput").ap()
    v = nc.dram_tensor("v", [PAIRS_PER_CORE, S, D], f32,
                       kind="ExternalInput").ap()
    o = nc.dram_tensor("o", [PAIRS_PER_CORE, S, D], f32,
                       kind="ExternalOutput").ap()
    with tile.TileContext(nc) as tc:
        build_attention(nc, tc, q, k, v, o, repeat_loop=repeat_loop,
                        mode=mode)
    nc.compile()
    _CACHED[key] = nc
    return nc


def kernel(queries, keys, values, adj=None, **_unused):
    """Full-input attention on 8 NeuronCores. Returns [S, B, H, D] fp32."""
    queries = np.ascontiguousarray(queries, dtype=np.float32)
    keys = np.ascontiguousarray(keys, dtype=np.float32)
    values = np.ascontiguousarray(values, dtype=np.float32)

    nc = build_program()
    qf = queries.reshape(B * H, S, D)
    kf = keys.reshape(B * H, S, D)
    vf = values.reshape(B * H, S, D)
    in_maps = []
    for c in range(N_CORES):
        sl = slice(c * PAIRS_PER_CORE, (c + 1) * PAIRS_PER_CORE)
        in_maps.append({"q": qf[sl], "k": kf[sl], "v": vf[sl]})
    res = run_bass_kernel_spmd(nc, in_maps, list(range(N_CORES)))
    hout = np.empty((B * H, S, D), dtype=np.float32)
    for c in range(N_CORES):
        hout[c * PAIRS_PER_CORE:(c + 1) * PAIRS_PER_CORE] = res.results[c]["o"]
    return hout.reshape(B, H, S, D).reshape(S, B, H, D)


# revision 18
# speedup vs baseline: 1.1187x; 1.1011x over previous
"""Bass/Trainium2 kernel for nn_DotProductAttention_47528108097846.

reference:
    scores = einsum('bhqd,bhkd->bhqk', Q, K) / 16
    attn = softmax(scores, axis=-1)
    h = einsum('bhqk,bhkd->bhqd', attn, V)
    return reshape(h, (S, B, H, D))

B=2, H=8, S=4096, D=64. 16 (b,h) pairs sharded as 2 per NeuronCore across 8
cores (batch+head parallel, no cross-core comms).

Per-core algorithm (2 heads), all matmuls bf16, fp32 PSUM accumulation:

  prologue (DMA/xbar only -- no PE, no DVE):
  - Q/K/V loaded with gpsimd-initiated casting DMAs (fp32 DRAM -> bf16
    SBUF), 128-row blocks in partitions.
  - K: per pair of 128-key blocks (2p, 2p+1), one XBAR dma_start_transpose
    of the [128, 128] pair view yields kt[:, p, :]: rows 0:64 = D dims of
    block 2p, rows 64:128 = D dims of block 2p+1 -- a full-128-contraction
    weight tile serving both blocks.
  - Q: same XBAR, then split by parity into qt_lo (even q-blocks in rows
    0:64, rows 64:128 zero) and qt_hi (odd q-blocks in rows 64:128, rows
    0:64 zero). The zero half makes the unwanted parity of the K pair-tile
    contract to 0.
  - V' = [V | 1 | 0pad] [128, 128] per block (ones column 64 gives the
    softmax denominator for free in the AV matmul).

  main loop, per q-group (1024 q) x k-block (128 keys):
  - scoresT j-half [128, 512] = (lhsT=kt[:, kb//2, :]).T @ (rhs = qt_lo or
    qt_hi 4-block slice).  j=0 streams the 4 even q-blocks, j=1 the 4 odd
    ones (a per-q-group column permutation undone by the output DMA).
  - exp: bank A (j=0) on ScalarE (exp activation, scale 1/16, bias
    16*ln(d0)); bank B (j=1) on the DVE via a custom 8-stage op
    ((c0*s+c1)*s+1)^16 ~= exp(s/16)/d0^16 (deg-2 fit + 4 squarings).
    Softmax cancels the shared d0^16.  Separate single-bank PSUM pools keep
    the two QK->exp->QK chains independent.
  - outT [128,1024] += (lhsT=V'_kb).T @ expT, software-pipelined at depth 2
    (AV of kb-2 interleaves QK of kb) so exp latency hides behind ~1.7us of
    in-order PE work.
  - epilogue: copy outT[0:65] to SBUF (DVE), PE-transpose [65,128] strips,
    reciprocal of the denominator column (DVE), scale-by-reciprocal
    (ScalarE activation Copy), permuted-destination DMA back to DRAM.
"""
import numpy as np

import concourse.bass as bass
import concourse.bacc as bacc
import concourse.tile as tile
from concourse import mybir
from concourse.masks import make_identity
from concourse.bass_utils import run_bass_kernel_spmd

B, H, S, D = 2, 8, 4096, 64
N_CORES = 8
PAIRS_PER_CORE = (B * H) // N_CORES  # 2 heads per core

f32 = mybir.dt.float32
bf16 = mybir.dt.bfloat16

QG = 1024            # q-group width
NQG = S // QG        # 4 q-groups per head
NKB = S // 128       # 32 k-blocks per head
NPB = NKB // 2       # 16 block pairs

# ---------------------------------------------------------------------------
# Custom DVE op: EXP16 -- out = ((c0*s + c1)*s + 1)^16 ~= exp(s/16)/d0^16.
# Deg-2 least-squares fit of e^u/d0 on u = s/256 in [-0.22, 0.22] (covers
# |s| <= 56; randn scores have sigma = 8).
EXP16_NAME = "EXP16_POLY_ANT"
EXP_D0 = 1.0000875648796109
EXP_E1 = 1.0070340603478836
EXP_E2 = 0.49672662859727144
EXP_C0 = float(EXP_E2 / 256.0**2)
EXP_C1 = float(EXP_E1 / 256.0)
EXP_BIAS = float(16.0 * np.log(EXP_D0))


def _np_exp16(in0, in1, s0, s1, imm2):
    q = (in0.astype(np.float32) * s0 + s1) * in0 + 1.0
    q = q * q
    q = q * q
    q = q * q
    return q * q


def register_exp16():
    import concourse.dve_ops as dve_ops_mod
    from concourse.dve_ops import DveOp
    from concourse.dve_spec import C0, C1, One, Spec, Src0, lower, _has_src1
    from concourse.dve_uop import DveOpSpec

    for op in dve_ops_mod.OPS:
        if op.name == EXP16_NAME:
            return op
    m1 = Src0 * C0
    a1 = m1 + C1
    m2 = a1 * Src0
    a2 = m2 + One
    y1 = a2 * a2
    y2 = y1 * y1
    y3 = y2 * y2
    y4 = y3 * y3
    spec = Spec(body=y4, reference=_np_exp16)
    row = dve_ops_mod._CUSTOM_DVE_ROW_BASE + len(dve_ops_mod.OPS)
    assert row < 0x20, "no free custom-DVE rows"
    dve_ops_mod._SUB_OPCODE_FOR_NAME[EXP16_NAME] = row
    shas = {}
    for ver in ("v3", "v4"):
        try:
            uops = lower(spec, ver=ver)
        except Exception:
            continue
        shas[ver] = DveOpSpec(
            name=EXP16_NAME, opcode=row, uops=uops, rd1_en=_has_src1(spec)
        ).sha(ver)
    op = DveOp(EXP16_NAME, spec, subdim=False, uops_sha=shas)
    dve_ops_mod.OPS.append(op)
    dve_ops_mod.CUSTOM_DVE_SPECS[EXP16_NAME] = spec
    return op


def build_attention(nc, tc, q, k, v, o, repeat_loop=None, mode="full"):
    """Emit attention for PAIRS_PER_CORE heads.

    q/k/v/o: DRAM APs of shape [PAIRS_PER_CORE, S, D] (fp32).
    """
    import contextlib
    exp16 = register_exp16()
    ctx = contextlib.ExitStack()
    consts = ctx.enter_context(tc.tile_pool(name="consts", bufs=1))
    nat = ctx.enter_context(tc.tile_pool(name="nat", bufs=2))
    natk = ctx.enter_context(tc.tile_pool(name="natk", bufs=2))
    natq = ctx.enter_context(tc.tile_pool(name="natq", bufs=2))
    natp = ctx.enter_context(tc.tile_pool(name="natp", bufs=2))
    persist = ctx.enter_context(tc.tile_pool(name="persist", bufs=1))
    sb = ctx.enter_context(tc.tile_pool(name="sb", bufs=3))
    pool_e = ctx.enter_context(tc.tile_pool(name="sb_e", bufs=6))
    # two independent single-bank score pools: PSUM pool rotation is
    # tile-granular, so bank A's reuse must not wait on bank B's reader
    pool_sA = ctx.enter_context(tc.tile_pool(name="ps_sA", bufs=2, space="PSUM"))
    pool_sB = ctx.enter_context(tc.tile_pool(name="ps_sB", bufs=2, space="PSUM"))
    pool_o = ctx.enter_context(tc.tile_pool(name="ps_o", bufs=1, space="PSUM"))
    pool_t = ctx.enter_context(tc.tile_pool(name="ps_t", bufs=2, space="PSUM"))

    if mode == "copyonly":
        for h in range(PAIRS_PER_CORE):
            t = None
            for src in (q, k, v):
                t = nat.tile([128, NKB, 64], f32, tag="nat")
                nc.sync.dma_start(
                    out=t, in_=src[h].rearrange("(n p) d -> p n d", p=128))
            nc.sync.dma_start(
                out=o[h].rearrange("(n p) d -> p n d", p=128), in_=t)
        ctx.close()
        return

    # exp bias for ScalarE (matches the DVE poly's d0^16 scale) + act-table
    # preload off the critical path
    bias_ap = consts.tile([128, 1], f32)
    nc.vector.memset(bias_ap, EXP_BIAS)
    dummy = consts.tile([128, 1], f32)
    nc.vector.memset(dummy, 0.0)
    dummy_o = consts.tile([128, 1], bf16)
    nc.scalar.activation(out=dummy_o, in_=dummy,
                         func=mybir.ActivationFunctionType.Exp,
                         bias=bias_ap, scale=1.0 / 16.0)

    ident = consts.tile([128, 128], f32)
    make_identity(nc, ident)

    # ---------------- prologue: casting DMAs + XBAR transposes ----------
    # Four zero-padded Q layouts: the kt pair tile holds K block 2p in
    # contraction rows 0:64 and block 2p+1 in rows 64:128, so scoring
    # k-block parity pk against q-parity pq needs the q content in rows
    # [64*pk : 64*pk+64] with the other half zero:
    #   ql_e: even q @ rows 0:64   ql_o: odd q @ rows 0:64
    #   qh_e: even q @ rows 64:128 qh_o: odd q @ rows 64:128
    qles, qlos_, qhes, qhos, kts, v1s = [], [], [], [], [], []

    def alloc_head(h):
        kt = persist.tile([128, NPB, 128], bf16, tag=f"kt{h}")
        ql_e = persist.tile([128, NPB, 128], bf16, tag=f"qle{h}")
        ql_o = persist.tile([128, NPB, 128], bf16, tag=f"qlo{h}")
        qh_e = persist.tile([128, NPB, 128], bf16, tag=f"qhe{h}")
        qh_o = persist.tile([128, NPB, 128], bf16, tag=f"qho{h}")
        v1 = persist.tile([128, NKB, 128], bf16, tag=f"v1{h}")
        kts.append(kt)
        qles.append(ql_e)
        qlos_.append(ql_o)
        qhes.append(qh_e)
        qhos.append(qh_o)
        v1s.append(v1)
        # zero parity halves (no other writer touches them).  Head 0's gate
        # the first matmuls -> idle DVE; later heads' go to gpsimd.
        ms = nc.vector if h == 0 else nc.gpsimd
        ms.memset(ql_e[64:128], 0.0)
        ms.memset(ql_o[64:128], 0.0)
        ms.memset(qh_e[0:64], 0.0)
        ms.memset(qh_o[0:64], 0.0)
        nc.gpsimd.memset(v1[:, :, 64:65], 1.0)
        nc.gpsimd.memset(v1[:, :, 65:128], 0.0)

    def emit_loads(h, b0, nb):
        """Load+transpose blocks [b0, b0+nb) of head h's K and Q."""
        kt = kts[h]
        ql_e, ql_o = qles[h], qlos_[h]
        qh_e, qh_o = qhes[h], qhos[h]
        gs = slice(b0 // 2, (b0 + nb) // 2)
        natbk = natk.tile([128, nb, 64], bf16, tag="natbk")
        nc.gpsimd.dma_start(
            out=natbk,
            in_=k[h].rearrange("(n p) d -> p n d", p=128)[:, b0:b0 + nb, :])
        # one batched XBAR: nb/2 pair-transposes per instruction
        nc.sync.dma_start_transpose(out=kt[:, gs, :], in_=natbk)
        natbq = natq.tile([128, nb, 64], bf16, tag="natbq")
        nc.gpsimd.dma_start(
            out=natbq,
            in_=q[h].rearrange("(n p) d -> p n d", p=128)[:, b0:b0 + nb, :])
        qtp = natp.tile([128, nb // 2, 128], bf16, tag="qtp")
        nc.sync.dma_start_transpose(out=qtp, in_=natbq)
        # all four parity-layout writes as SBUF->SBUF DMAs (two of them
        # cross partitions, which compute engines cannot do)
        nc.sync.dma_start(out=ql_e[0:64, gs, :], in_=qtp[0:64])
        nc.sync.dma_start(out=qh_o[64:128, gs, :], in_=qtp[64:128])
        nc.sync.dma_start(out=ql_o[0:64, gs, :], in_=qtp[64:128])
        nc.sync.dma_start(out=qh_e[64:128, gs, :], in_=qtp[0:64])

    def emit_vload(h):
        nc.gpsimd.dma_start(
            out=v1s[h][:, :, 0:64],
            in_=v[h].rearrange("(n p) d -> p n d", p=128))

    # all prologues upfront: they only use gpsimd + SP + DMA engines, which
    # the main loop leaves idle (out-DMAs are issued from ScalarE).  Head 0's
    # first chunk goes first for latency; all casting-DMA descriptor gens
    # (gpsimd) are emitted before any engine gets busy.
    for h in range(PAIRS_PER_CORE):
        alloc_head(h)
    emit_loads(0, 0, 8)
    emit_loads(0, 8, 24)
    emit_vload(0)
    for h in range(1, PAIRS_PER_CORE):
        emit_loads(h, 0, NKB)
        emit_vload(h)

    # ---------------- main loops --------------------------------------
    def main_compute():
        for h in range(PAIRS_PER_CORE):
            kt, v1 = kts[h], v1s[h]
            ql_e, ql_o, qh_e, qh_o = qles[h], qlos_[h], qhes[h], qhos[h]
            out_r = o[h].rearrange("(n p) d -> p n d", p=128)
            for qg in range(NQG):
                ps_o = pool_o.tile([128, QG], f32, tag="o")

                def av(prev_eT, prev_kb, j):
                    nc.tensor.matmul(
                        out=ps_o[:, j * 512:(j + 1) * 512],
                        lhsT=v1[:, prev_kb, :],
                        rhs=prev_eT[:, j * 512:(j + 1) * 512],
                        start=(prev_kb == 0), stop=(prev_kb == NKB - 1))

                # software-pipelined at depth 2: QK(kb) interleaves with the
                # accumulating AV matmuls of kb-2 (the PE runs its queue in
                # order, so AV readiness must trail by ~2 tiles of PE work)
                pend = []
                for kb in range(NKB):
                    ps_sa = pool_sA.tile([128, 512], f32, tag="sA")
                    ps_sb = pool_sB.tile([128, 512], f32, tag="sB")
                    ps_sj = (ps_sa, ps_sb)
                    eT = pool_e.tile([128, QG], bf16, tag="exp")
                    qj = (ql_e, ql_o) if kb % 2 == 0 else (qh_e, qh_o)
                    for j in range(QG // 512):
                        nc.tensor.matmul(
                            out=ps_sj[j],
                            lhsT=kt[:, kb // 2, :],
                            rhs=qj[j][:, 4 * qg:4 * qg + 4, :],
                            start=True, stop=True)
                        # exp of this half right after its producing matmul:
                        # bank A (even q-blocks) -> ScalarE, bank B (odd) ->
                        # DVE custom op
                        if j == 0:
                            nc.scalar.activation(
                                out=eT[:, 0:512], in_=ps_sj[0],
                                func=mybir.ActivationFunctionType.Exp,
                                bias=bias_ap, scale=1.0 / 16.0)
                        else:
                            nc.vector._custom_dve(
                                exp16, out=eT[:, 512:QG], in0=ps_sj[1],
                                s0=EXP_C0, s1=EXP_C1)
                        if len(pend) >= 2:
                            av(pend[0][0], pend[0][1], j)
                            if j == QG // 512 - 1:
                                pend.pop(0)
                    pend.append((eT, kb))
                for eT_p, kb_p in pend:
                    for j in range(QG // 512):
                        av(eT_p, kb_p, j)

                # epilogue for this q-group.  ps_o columns hold q-blocks in
                # parity-permuted order [0,2,4,6,1,3,5,7]; the DMA dest APs
                # restore natural order.
                oT = sb.tile([65, QG], f32, tag="oT")
                nc.vector.tensor_copy(out=oT, in_=ps_o[0:65, :])
                out_sb = sb.tile([128, QG // 128, 64], f32, tag="out")
                # dest AP [p, u, t, d]: (u, t) -> physical q-block 2t+u, so
                # iterating u-major hits [0,2,4,6,1,3,5,7] -- undoing the
                # parity permutation of out_sb's i index (i = u*4+t)
                dst8 = out_r[:, qg * 8:(qg + 1) * 8, :].rearrange(
                    "p (t u) d -> p u t d", t=4, u=2)
                src8 = None  # filled per half below
                for i in range(QG // 128):
                    ps_t = pool_t.tile([128, 65], f32, tag="t")
                    nc.tensor.transpose(
                        ps_t, oT[:, i * 128:(i + 1) * 128],
                        ident[0:65, 0:65])
                    rcp = sb.tile([128, 1], f32, tag="rcp")
                    nc.vector.reciprocal(out=rcp, in_=ps_t[:, 64:65])
                    nc.scalar.activation(
                        out=out_sb[:, i, :], in_=ps_t[:, 0:64],
                        func=mybir.ActivationFunctionType.Copy,
                        scale=rcp)
                    if i == 3:
                        nc.scalar.dma_start(
                            out=dst8[:, 0, :, :], in_=out_sb[:, 0:4, :])
                nc.scalar.dma_start(
                    out=dst8[:, 1, :, :], in_=out_sb[:, 4:8, :])

    if mode == "prologue":
        pass
    elif repeat_loop is None:
        main_compute()
    else:
        with tc.For_i(0, repeat_loop, 1) as _:
            main_compute()
    ctx.close()


_CACHED = {}


def build_program(repeat_loop=None, mode="full"):
    key = (repeat_loop, mode)
    if key in _CACHED:
        return _CACHED[key]
    nc = bacc.Bacc("TRN2", target_bir_lowering=False, debug=False,
                   num_devices=N_CORES)
    q = nc.dram_tensor("q", [PAIRS_PER_CORE, S, D], f32,
                       kind="ExternalInput").ap()
    k = nc.dram_tensor("k", [PAIRS_PER_CORE, S, D], f32,
                       kind="ExternalInput").ap()
    v = nc.dram_tensor("v", [PAIRS_PER_CORE, S, D], f32,
                       kind="ExternalInput").ap()
    o = nc.dram_tensor("o", [PAIRS_PER_CORE, S, D], f32,
                       kind="ExternalOutput").ap()
    with tile.TileContext(nc) as tc:
        build_attention(nc, tc, q, k, v, o, repeat_loop=repeat_loop,
                        mode=mode)
    nc.compile()
    _CACHED[key] = nc
    return nc


def kernel(queries, keys, values, adj=None, **_unused):
    """Full-input attention on 8 NeuronCores. Returns [S, B, H, D] fp32."""
    queries = np.ascontiguousarray(queries, dtype=np.float32)
    keys = np.ascontiguousarray(keys, dtype=np.float32)
    values = np.ascontiguousarray(values, dtype=np.float32)

    nc = build_program()
    qf = queries.reshape(B * H, S, D)
    kf = keys.reshape(B * H, S, D)
    vf = values.reshape(B * H, S, D)
    in_maps = []
    for c in range(N_CORES):
        sl = slice(c * PAIRS_PER_CORE, (c + 1) * PAIRS_PER_CORE)
        in_maps.append({"q": qf[sl], "k": kf[sl], "v": vf[sl]})
    res = run_bass_kernel_spmd(nc, in_maps, list(range(N_CORES)))
    hout = np.empty((B * H, S, D), dtype=np.float32)
    for c in range(N_CORES):
        hout[c * PAIRS_PER_CORE:(c + 1) * PAIRS_PER_CORE] = res.results[c]["o"]
    return hout.reshape(B, H, S, D).reshape(S, B, H, D)
